# revision 40
# baseline (speedup 1.0000x reference)
"""L-BFGS two-loop recursion (apply_Hv) on 8 Trainium2 NeuronCores.

Vector-free reformulation (two streaming passes + tiny scalar recursion):

  1. Gram pass  : G = [S; Y; av] @ [Y; av]^T.  Inputs are cast fp32->fp8
                  (xSC) once on DVE; fp8 PE transposes (identity matmul,
                  stride-2 PSUM writes) + one merged 62-col fp8 Gram matmul
                  per 128-sample block, accumulated in PSUM across the whole
                  pass.  The natural-layout fp8 cast of s,y is also written
                  to DRAM scratch (30 MB/core) for the combine pass.
  2. AllReduce  : 7.6 KB partial-Gram merge across the 8 cores.
  3. recursion  : alpha/beta scalar recursions on the 61x31 Gram (serial
                  DVE ops, ~16us).
  4. combine    : res = (a/theta) v + sum_j c_s[j] s_j + c_y[j] y_j; fp8
                  rows streamed from scratch; 70% of rows are multiplied on
                  the Scalar engine into tmps that DVE adds, the rest are
                  fused multiply-adds on DVE.

Layout: two 4096-sample halves stacked at partition bases 0 and 64 (rows
61-63/125-127 padded with junk v data so engine slices stay at legal
partition bases 0/64).

HBM traffic/core: 126 MB fp32 in + 30 MB fp8 out (pass 1), 32 MB fp8+fp32
in + 2 MB out (pass 2) = ~192 MB vs 256 MB for an all-fp32 two-pass.
fp8 e4m3 end-to-end rel err vs the fp32 reference: ~1.8e-3 (tol 2e-2).
"""

import os

import numpy as np

import concourse.bass as bass
import concourse.mybir as mybir
from concourse import bacc
from concourse.bass_utils import run_bass_kernel_spmd
from concourse.masks import make_identity
from concourse.tile import TileContext

F32 = mybir.dt.float32
F8 = mybir.dt.float8e4
M = 30          # L-BFGS history length
X = 2 * M + 1   # rows of [S; Y; v]
PH = 64         # padded rows per half
W = M + 1       # cols of [Y; v]
NCORES = 8
N_FULL = 4_194_304
N_CORE = N_FULL // NCORES
SC = 32.0       # uniform fp8 pre-scale (32*|v|max ~ 170 < fp8 e4m3 max)

F_A = 4096
CH_A = 2 * F_A

# phase D: fraction of rows handled as DVE-direct fused multiply-adds; the
# rest are scalar-engine multiplies with DVE adds.
D_DIRECT = int(os.environ.get("D_DIRECT", "3"))  # direct if r % 10 < D_DIRECT


def build_kernel(n_core: int = N_CORE, n_cores: int = NCORES):
    assert n_core % CH_A == 0
    n_chunks_a = n_core // CH_A
    f_d = min(2048, n_core // 128)
    ch_d = 128 * f_d
    assert n_core % ch_d == 0
    n_chunks_d = n_core // ch_d

    nc = bacc.Bacc(None, target_bir_lowering=False, debug=False)

    s_d = nc.declare_dram_parameter("s_s", [M, n_core], F32, isOutput=False)
    y_d = nc.declare_dram_parameter("y_s", [M, n_core], F32, isOutput=False)
    v_d = nc.declare_dram_parameter("v_s", [n_core], F32, isOutput=False)
    ys_d = nc.declare_dram_parameter("ys", [M], F32, isOutput=False)
    th_d = nc.declare_dram_parameter("theta", [1], F32, isOutput=False)
    a_d = nc.declare_dram_parameter("a", [1], F32, isOutput=False)
    out_d = nc.declare_dram_parameter("out", [n_core], F32, isOutput=True)

    scr_d = nc.dram_tensor("scr", [2 * M, n_core], F8)  # fp8 copies of s, y
    g_loc = nc.dram_tensor("g_loc", [X, W], F32)
    g_red = nc.dram_tensor("g_red", [X, W], F32, addr_space="Shared")

    add = mybir.AluOpType.add
    mult = mybir.AluOpType.mult

    with TileContext(nc) as tc:
        with (
            tc.tile_pool(name="consts", bufs=1) as consts,
            tc.tile_pool(name="x2", bufs=3) as x2_pool,
            tc.tile_pool(name="x8", bufs=3) as x8_pool,
            tc.tile_pool(name="xt", bufs=4) as xt_pool,
            tc.tile_pool(name="pstr", bufs=4, space="PSUM") as pstr_pool,
            tc.tile_pool(name="psg", bufs=1, space="PSUM") as psg_pool,
            tc.tile_pool(name="psmisc", bufs=1, space="PSUM") as psmisc_pool,
            tc.tile_pool(name="small", bufs=1) as small,
            tc.tile_pool(name="dacc", bufs=2) as dacc_pool,
            tc.tile_pool(name="dtmp", bufs=4) as dtmp_pool,
            tc.tile_pool(name="drow", bufs=20) as drow_pool,
            tc.tile_pool(name="dv", bufs=2) as dv_pool,
        ):
            # ---------------- constants / start-of-kernel precompute ------
            id32 = consts.tile([128, 128], F32)
            make_identity(nc, id32)
            identity = consts.tile([128, 128], F8)
            nc.vector.tensor_copy(identity, id32)
            ones128 = consts.tile([1, 128], F32)
            nc.vector.memset(ones128, 1.0)

            ys_t = small.tile([1, M], F32)
            nc.sync.dma_start(out=ys_t, in_=ys_d[:].rearrange("(o a) -> o a", o=1))
            a_t = small.tile([1, 1], F32)
            nc.sync.dma_start(out=a_t, in_=a_d[:].rearrange("(o a) -> o a", o=1))
            th_t = small.tile([1, 1], F32)
            nc.sync.dma_start(out=th_t, in_=th_d[:].rearrange("(o a) -> o a", o=1))
            inv_ys = small.tile([1, M], F32)
            nc.vector.reciprocal(inv_ys, ys_t)
            inv_th = small.tile([1, 1], F32)
            nc.vector.reciprocal(inv_th, th_t)

            # The gram fp8 un-scale matrix is rank-1: u_r (x) w_c with the
            # `a` factor on v folded into w_c's last column.  Fold u_r into
            # the row normalizers, w_c into a PE-broadcast [X, W] tile.
            rr = small.tile([1, X], F32)
            nc.vector.tensor_scalar(
                out=rr[:, 0:M], in0=inv_ys, scalar1=1.0 / SC, scalar2=None, op0=mult
            )
            nc.vector.tensor_scalar(
                out=rr[:, M : 2 * M], in0=inv_ys, scalar1=inv_th,
                scalar2=1.0 / SC, op0=mult, op1=mult,
            )
            nc.vector.memset(rr[:, 2 * M : X], 1.0)
            one1 = small.tile([1, 1], F32)
            nc.vector.memset(one1, 1.0)
            ps_rn = psmisc_pool.tile([128, X], F32, tag="pmisc")
            nc.tensor.matmul(ps_rn[0:X, 0:1], rr, one1, start=True, stop=True)
            rownorm = small.tile([X, 1], F32)
            nc.vector.tensor_copy(rownorm, ps_rn[0:X, 0:1])

            # -inv_ys/SC^2 broadcast to [M, M] rows (for the u/e recursion)
            neg_invys = small.tile([1, M], F32)
            nc.vector.tensor_scalar(
                out=neg_invys, in0=inv_ys, scalar1=-1.0 / (SC * SC), scalar2=None,
                op0=mult,
            )
            ps_bc = psmisc_pool.tile([128, X], F32, tag="pmisc")
            nc.tensor.matmul(
                ps_bc[0:M, 0:M], ones128[:, 0:M], neg_invys, start=True, stop=True
            )
            bc_niy = small.tile([M, M], F32)
            nc.vector.tensor_copy(bc_niy, ps_bc[0:M, 0:M])

            # w_c column un-scale broadcast to [X, W]: 1/SC on y cols, a/SC
            # on the v col (v is fp8-scaled by SC like everything else)
            wrow = small.tile([1, W], F32)
            nc.vector.memset(wrow[:, 0:M], 1.0 / SC)
            nc.vector.tensor_scalar(
                out=wrow[:, M : M + 1], in0=a_t, scalar1=1.0 / SC, scalar2=None,
                op0=mult,
            )
            ps_wb = psmisc_pool.tile([128, X], F32, tag="pmisc")
            nc.tensor.matmul(ps_wb[0:X, 0:W], ones128[:, 0:X], wrow, start=True, stop=True)
            wbc = small.tile([X, W], F32)
            nc.vector.tensor_copy(wbc, ps_wb[0:X, 0:W])

            # ---------------- phase A: Gram + fp8 scratch -----------------
            n_blk = F_A // 128
            n_grp = n_blk // 4  # 4-transpose groups per chunk
            SKEW = 2            # matmuls run SKEW groups behind transposes
            total_mm = n_chunks_a * n_blk

            # one PSUM tile accumulates the whole Gram pass
            gps = psg_pool.tile([128, 2, W], F32, tag="gps")

            def emit_matmul(xt, t, mmi):
                # lhsT = full 128-col transposed block; rhs = the [Y; v]
                # columns of both halves -> out [128, 2, W]
                nc.tensor.matmul(
                    gps,
                    xt[:, t, :],
                    xt.rearrange("p g (h c) -> p g h c", h=2)[:, t, :, M : M + W],
                    start=(mmi == 0),
                    stop=(mmi == total_mm - 1),
                )

            # software pipeline across chunks: DMAs issued 2 chunks ahead,
            # the fp8 cast 1 chunk ahead — so the DVE never head-of-line
            # blocks on a wait (its stream is cast(c+1) then copies(c),
            # both of whose inputs are already complete).
            def issue_dmas(c):
                n0 = c * CH_A
                x2 = x2_pool.tile([128, F_A], F32, tag="x2")
                for h in range(2):
                    nh = n0 + h * F_A
                    nc.sync.dma_start(
                        out=x2[h * PH : h * PH + M, :], in_=s_d[:, nh : nh + F_A]
                    )
                    nc.sync.dma_start(
                        out=x2[h * PH + M : h * PH + 2 * M, :],
                        in_=y_d[:, nh : nh + F_A],
                    )
                    nc.sync.dma_start(
                        out=x2[h * PH + 2 * M : h * PH + X, :],
                        in_=v_d[nh : nh + F_A].rearrange("(o f) -> o f", o=1),
                    )
                    # junk pad rows (finite data, never read downstream)
                    nc.sync.dma_start(
                        out=x2[h * PH + X : h * PH + PH, :],
                        in_=v_d[0 : (PH - X) * F_A].rearrange(
                            "(p f) -> p f", p=PH - X
                        ),
                    )
                return x2

            def cast_chunk(x2):
                x8 = x8_pool.tile([128, F_A], F8, tag="x8")
                nc.vector.tensor_scalar(
                    out=x8, in0=x2, scalar1=SC, scalar2=None, op0=mult
                )
                return x8

            def process_chunk(c, x8):
                n0 = c * CH_A
                nc.gpsimd.dma_start(
                    out=scr_d[:, n0 : n0 + F_A], in_=x8[0 : 2 * M, :]
                )
                nc.gpsimd.dma_start(
                    out=scr_d[:, n0 + F_A : n0 + CH_A],
                    in_=x8[PH : PH + 2 * M, :],
                )
                # PE: fp8 transposes (stride-2 PSUM); merged Gram matmuls
                # run SKEW groups behind so PE never waits on the copies
                pending = []
                for q in range(n_grp):
                    ps = pstr_pool.tile([128, 4, 128, 2], F8, tag="pstr")
                    for t in range(4):
                        b = q * 4 + t
                        nc.tensor.transpose(
                            ps[:, t, :, 0], x8[:, b * 128 : (b + 1) * 128], identity
                        )
                    xt = xt_pool.tile([128, 4, 128], F8, tag="xt")
                    # PSUM->SBUF copies: 6/8 scalar, 2/8 vector
                    if q % 8 < 6:
                        nc.scalar.copy(xt[:, :, :], ps[:, :, :, 0])
                    else:
                        nc.vector.tensor_copy(xt[:, :, :], ps[:, :, :, 0])
                    pending.append(xt)
                    if q >= SKEW:
                        for t in range(4):
                            emit_matmul(pending[q - SKEW], t, c * n_blk + (q - SKEW) * 4 + t)
                for q in range(n_grp - SKEW, n_grp):
                    for t in range(4):
                        emit_matmul(pending[q], t, c * n_blk + q * 4 + t)

            x2_cur = issue_dmas(0)
            x2_next = issue_dmas(1) if n_chunks_a > 1 else None
            x8_cur = cast_chunk(x2_cur)
            for c in range(n_chunks_a):
                x2_next2 = issue_dmas(c + 2) if c + 2 < n_chunks_a else None
                x8_next = cast_chunk(x2_next) if x2_next is not None else None
                process_chunk(c, x8_cur)
                x2_next, x8_cur = x2_next2, x8_next

            # fold the two halves of the accumulated Gram: [61, 31]
            # (only one PSUM operand allowed per instruction)
            g_h0 = small.tile([X, W], F32)
            nc.vector.tensor_copy(g_h0, gps[0:X, 0, :])
            g_acc = small.tile([X, W], F32)
            nc.vector.tensor_tensor(
                out=g_acc, in0=g_h0, in1=gps[PH : PH + X, 1, :], op=add
            )
            # phase B/C DMAs go through the VECTOR DMA queue so the sync
            # queue's phase-D prefetch is not head-of-line blocked behind
            # the collective dependency
            nc.scalar.dma_start(out=g_loc[:, :], in_=g_acc)
            nc.gpsimd.collective_compute(
                "AllReduce",
                add,
                ins=[g_loc[:, :]],
                outs=[g_red[:, :]],
                replica_groups=[list(range(n_cores))],
            )

            # ---------------- phase C: scalar recursions ------------------
            gf2d = small.tile([X, W], F32)
            nc.scalar.dma_start(out=gf2d, in_=g_red[:, :])
            gn2d = small.tile([X, W], F32)
            nc.vector.tensor_scalar(
                out=gn2d, in0=gf2d, scalar1=rownorm, scalar2=None, op0=mult
            )
            nc.vector.tensor_tensor(out=gn2d, in0=gn2d, in1=wbc, op=mult)
            sc2d = small.tile([M, M], F32)
            nc.vector.tensor_tensor(out=sc2d, in0=gf2d[0:M, 0:M], in1=bc_niy, op=mult)
            gnf = small.tile([1, X * W], F32)
            nc.scalar.dma_start(out=gnf, in_=gn2d[:, :])
            scf = small.tile([1, M * M], F32)
            nc.scalar.dma_start(out=scf, in_=sc2d[:, :])

            gnf_r = gnf.rearrange("o (r c) -> o r c", c=W)
            junk = small.tile([1, M], F32)
            alpha = small.tile([1, M], F32)
            nc.vector.tensor_copy(alpha, gnf_r[:, 0:M, M])
            dotn = small.tile([1, 1], F32)
            # loop 1 (serial): alpha_j += -sum_{k>j} SYn[j,k] alpha_k
            for j in range(M - 2, -1, -1):
                nk = M - 1 - j
                nc.vector.tensor_tensor(
                    out=junk[:, 0:nk],
                    in0=gnf[:, j * W + j + 1 : j * W + M],
                    in1=alpha[:, j + 1 : M],
                    op=mult,
                )
                nc.vector.tensor_reduce(
                    out=dotn, in_=junk[:, 0:nk],
                    axis=mybir.AxisListType.X, op=add, negate=True,
                )
                nc.vector.tensor_tensor(
                    out=alpha[:, j : j + 1], in0=alpha[:, j : j + 1], in1=dotn, op=add
                )
            # w~_j = a*Yv_j/(ys_j th) - sum_k YYn[j,k] alpha_k  (independent)
            wv = small.tile([1, M], F32)
            nc.vector.tensor_copy(wv, gnf_r[:, M : 2 * M, M])
            dotw = small.tile([1, M], F32)
            for j in range(M):
                nc.vector.tensor_tensor(
                    out=junk,
                    in0=gnf[:, (M + j) * W : (M + j) * W + M],
                    in1=alpha,
                    op=mult,
                )
                nc.vector.tensor_reduce(
                    out=dotw[:, j : j + 1], in_=junk,
                    axis=mybir.AxisListType.X, op=add, negate=True,
                )
            nc.vector.tensor_tensor(out=wv, in0=wv, in1=dotw, op=add)
            # loop 2 (serial): dbar_j = w~_j + e_j ; e[k>j] += dbar_j*scf[j,k]
            ebar = small.tile([1, M], F32)
            nc.vector.tensor_scalar(
                out=ebar, in0=alpha, scalar1=-1.0, scalar2=None, op0=mult
            )
            dbar = small.tile([1, M], F32)
            for j in range(M):
                nc.vector.tensor_tensor(
                    out=dbar[:, j : j + 1],
                    in0=wv[:, j : j + 1],
                    in1=ebar[:, j : j + 1],
                    op=add,
                )
                if j < M - 1:
                    nc.vector.scalar_tensor_tensor(
                        out=ebar[:, j + 1 : M],
                        in0=scf[:, j * M + j + 1 : j * M + M],
                        scalar=dbar[:, j : j + 1],
                        in1=ebar[:, j + 1 : M],
                        op0=mult,
                        op1=add,
                    )
            # coefficients: c_s = -dbar/SC ; c_y = -alpha/(th*SC) ; c_v = a/th
            coeff = small.tile([1, X], F32)
            nc.vector.tensor_scalar(
                out=coeff[:, 0:M], in0=dbar, scalar1=-1.0 / SC, scalar2=None, op0=mult
            )
            nc.vector.tensor_scalar(
                out=coeff[:, M : 2 * M], in0=alpha, scalar1=inv_th,
                scalar2=-1.0 / SC, op0=mult, op1=mult,
            )
            nc.vector.tensor_scalar(
                out=coeff[:, 2 * M : X], in0=a_t, scalar1=inv_th, scalar2=None, op0=mult
            )
            cb_ps = psmisc_pool.tile([128, X], F32, tag="pmisc")
            nc.tensor.matmul(cb_ps, ones128, coeff, start=True, stop=True)
            c_full = small.tile([128, X], F32)
            nc.vector.tensor_copy(c_full, cb_ps)

            # ---------------- phase D: res = sum_j coeff_j * row_j --------
            for c in range(n_chunks_d):
                n0 = c * ch_d
                acc = dacc_pool.tile([128, f_d], F32, tag="dacc")
                vch = dv_pool.tile([128, f_d], F32, tag="dv")
                nc.sync.dma_start(
                    out=vch, in_=v_d[n0 : n0 + ch_d].rearrange("(p f) -> p f", p=128)
                )
                nc.vector.tensor_scalar(
                    out=acc, in0=vch, scalar1=c_full[:, X - 1 : X], scalar2=None,
                    op0=mult,
                )
                for r in range(2 * M):
                    row = drow_pool.tile([128, f_d], F8, tag="drow")
                    nc.sync.dma_start(
                        out=row,
                        in_=scr_d[r, n0 : n0 + ch_d].rearrange("(p f) -> p f", p=128),
                    )
                    if r % 10 < D_DIRECT:
                        nc.vector.scalar_tensor_tensor(
                            out=acc, in0=row, scalar=c_full[:, r : r + 1], in1=acc,
                            op0=mult, op1=add,
                        )
                    else:
                        tmp = dtmp_pool.tile([128, f_d], mybir.dt.bfloat16, tag="dtmp")
                        nc.scalar.activation(
                            tmp, row, mybir.ActivationFunctionType.Copy,
                            scale=c_full[:, r : r + 1],
                        )
                        nc.vector.tensor_tensor(out=acc, in0=acc, in1=tmp, op=add)
                nc.sync.dma_start(
                    out=out_d[n0 : n0 + ch_d].rearrange("(p f) -> p f", p=128),
                    in_=acc,
                )

    nc.compile()
    return nc


_BUILD_CACHE = {}


def _get_nc(n_core: int, n_cores: int):
    key = (n_core, n_cores)
    if key not in _BUILD_CACHE:
        _BUILD_CACHE[key] = build_kernel(n_core, n_cores)
    return _BUILD_CACHE[key]


def run(v, s, y, ys, theta, a, trace=False):
    n = v.shape[0]
    n_core = n // NCORES
    nc = _get_nc(n_core, NCORES)
    in_maps = []
    for c in range(NCORES):
        sl = slice(c * n_core, (c + 1) * n_core)
        in_maps.append(
            {
                "s_s": np.ascontiguousarray(s[:, sl]),
                "y_s": np.ascontiguousarray(y[:, sl]),
                "v_s": np.ascontiguousarray(v[sl]),
                "ys": np.ascontiguousarray(ys),
                "theta": np.asarray(theta, dtype=np.float32).reshape(1),
                "a": np.asarray(a, dtype=np.float32).reshape(1),
            }
        )
    res = run_bass_kernel_spmd(nc, in_maps, list(range(NCORES)), trace=trace)
    out = np.concatenate([res.results[c]["out"] for c in range(NCORES)])
    return out, res


def kernel(v, s, y, ys, theta, a):
    out, _ = run(
        np.asarray(v, np.float32),
        np.asarray(s, np.float32),
        np.asarray(y, np.float32),
        np.asarray(ys, np.float32),
        theta,
        a,
    )
    return out


# revision 41
# speedup vs baseline: 1.0929x; 1.0929x over previous
"""L-BFGS two-loop recursion (apply_Hv) on 8 Trainium2 NeuronCores.

Vector-free reformulation: instead of 60 sequential dot-product/axpy steps
(each of which would need its own scalar AllReduce at a ~10us floor), the
two-loop recursion is algebraically equivalent to

  1. Gram pass   : G = [S; Y; v] @ [Y; v]^T            (one streaming pass)
  2. tiny scalar : alpha/beta recursions on the 61x31 Gram entries
  3. combine pass: res = (a/theta) v - sum_j (alpha_j/theta) y_j
                       + sum_j (alpha_j - beta_j) s_j  (one streaming pass)

The n dimension is sharded across the 8 cores; one 7.6 KB AllReduce merges
the per-core partial Gram matrices.  Both passes are HBM-bandwidth bound.

Per core, phase A streams natural-layout [122, F] tiles (rows = S(30), Y(30),
v packed twice along partitions for full DMA width), transposes 128-column
blocks on the TensorEngine (fp32 needs the identity-matmul path), and
accumulates the Gram matrix in PSUM.  Phase D is a per-row fused
scalar_tensor_tensor accumulation on the VectorEngine.
"""

import numpy as np

import concourse.bass as bass
import concourse.mybir as mybir
from concourse import bacc
from concourse.bass_utils import run_bass_kernel_spmd
from concourse.masks import make_identity
from concourse.tile import TileContext

F32 = mybir.dt.float32
M = 30  # L-BFGS history length
X = 2 * M + 1  # rows of [S; Y; v]
NCORES = 8
N_FULL = 4_194_304
N_CORE = N_FULL // NCORES

# phase A: one chunk covers CH_A consecutive n per core, laid out as
# [122, F_A] (two n-halves stacked along partitions)
F_A = 4096
CH_A = 2 * F_A


def build_kernel(n_core: int = N_CORE, n_cores: int = NCORES):
    assert n_core % CH_A == 0
    n_chunks_a = n_core // CH_A
    # phase D: [128, f_d] tiles, n-chunk = 128 * f_d
    f_d = min(2048, n_core // 128)
    ch_d = 128 * f_d
    assert n_core % ch_d == 0
    n_chunks_d = n_core // ch_d

    nc = bacc.Bacc(None, target_bir_lowering=False, debug=False)

    s_d = nc.declare_dram_parameter("s_s", [M, n_core], F32, isOutput=False)
    y_d = nc.declare_dram_parameter("y_s", [M, n_core], F32, isOutput=False)
    v_d = nc.declare_dram_parameter("v_s", [n_core], F32, isOutput=False)
    ys_d = nc.declare_dram_parameter("ys", [M], F32, isOutput=False)
    th_d = nc.declare_dram_parameter("theta", [1], F32, isOutput=False)
    a_d = nc.declare_dram_parameter("a", [1], F32, isOutput=False)
    out_d = nc.declare_dram_parameter("out", [n_core], F32, isOutput=True)

    g_loc = nc.dram_tensor("g_loc", [X, M + 1], F32)
    g_red = nc.dram_tensor("g_red", [X, M + 1], F32, addr_space="Shared")

    add = mybir.AluOpType.add
    mult = mybir.AluOpType.mult
    subtract = mybir.AluOpType.subtract

    with TileContext(nc) as tc:
        with (
            tc.tile_pool(name="consts", bufs=1) as consts,
            tc.tile_pool(name="x2", bufs=3) as x2_pool,
            tc.tile_pool(name="xt", bufs=4) as xt_pool,
            tc.tile_pool(name="pstr", bufs=3, space="PSUM") as pstr_pool,
            tc.tile_pool(name="psg", bufs=2, space="PSUM") as psg_pool,
            tc.tile_pool(name="small", bufs=1) as small,
            tc.tile_pool(name="dacc", bufs=2) as dacc_pool,
            tc.tile_pool(name="drow", bufs=10) as drow_pool,
            tc.tile_pool(name="dv", bufs=2) as dv_pool,
        ):
            identity = consts.tile([122, 122], F32)
            make_identity(nc, identity)

            # ---------------- phase A: Gram matrix ----------------
            g_acc = small.tile([X, M + 1], F32)
            nc.vector.memset(g_acc, 0.0)

            for c in range(n_chunks_a):
                n0 = c * CH_A
                x2 = x2_pool.tile([2 * X, F_A], F32, tag="x2")
                # partition p = h*61 + j holds row j of [S;Y;v], n-half h.
                # plain partition-range slices only (nested partition dims in
                # one SBUF DMA dst are not supported).
                for h in range(2):
                    nh = n0 + h * F_A
                    nc.sync.dma_start(
                        out=x2[h * X : h * X + M, :],
                        in_=s_d[:, nh : nh + F_A],
                    )
                    nc.sync.dma_start(
                        out=x2[h * X + M : h * X + 2 * M, :],
                        in_=y_d[:, nh : nh + F_A],
                    )
                    nc.sync.dma_start(
                        out=x2[h * X + 2 * M : h * X + X, :],
                        in_=v_d[nh : nh + F_A].rearrange("(o f) -> o f", o=1),
                    )

                gps = psg_pool.tile([X, M + 1], F32, tag="gps")
                n_blk = F_A // 128  # 128-column transpose blocks
                mm = 0
                for q in range(n_blk // 4):
                    ps = pstr_pool.tile([128, 4, 2 * X], F32, tag="pstr")
                    for t in range(4):
                        b = q * 4 + t
                        nc.tensor.transpose(
                            ps[:, t, :], x2[:, b * 128 : (b + 1) * 128], identity
                        )
                    xt = xt_pool.tile([128, 4, 2 * X], F32, tag="xt")
                    nc.any.tensor_copy(xt, ps)
                    for t in range(4):
                        for h in range(2):
                            nc.tensor.matmul(
                                gps,
                                xt[:, t, h * X : (h + 1) * X],
                                xt[:, t, h * X + M : (h + 1) * X],
                                start=(mm == 0),
                                stop=(mm == 8 * (n_blk // 4) - 1),
                            )
                            mm += 1
                nc.vector.tensor_tensor(out=g_acc, in0=g_acc, in1=gps, op=add)

            # ---------------- phase B: AllReduce ----------------
            nc.sync.dma_start(out=g_loc[:, :], in_=g_acc)
            nc.gpsimd.collective_compute(
                "AllReduce",
                add,
                ins=[g_loc[:, :]],
                outs=[g_red[:, :]],
                replica_groups=[list(range(n_cores))],
            )

            # ---------------- phase C: scalar recursions ----------------
            # everything on partition 0; G flattened to [1, X*(M+1)]
            W = M + 1
            gf = small.tile([1, X * W], F32)
            nc.sync.dma_start(
                out=gf, in_=g_red[:, :].rearrange("(o a) b -> o (a b)", o=1)
            )
            ys_t = small.tile([1, M], F32)
            nc.sync.dma_start(out=ys_t, in_=ys_d[:].rearrange("(o a) -> o a", o=1))
            a_t = small.tile([1, 1], F32)
            nc.sync.dma_start(out=a_t, in_=a_d[:].rearrange("(o a) -> o a", o=1))
            th_t = small.tile([1, 1], F32)
            nc.sync.dma_start(out=th_t, in_=th_d[:].rearrange("(o a) -> o a", o=1))

            inv_ys = small.tile([1, M], F32)
            nc.vector.reciprocal(inv_ys, ys_t)
            inv_th = small.tile([1, 1], F32)
            nc.vector.reciprocal(inv_th, th_t)

            coeff = small.tile([1, 2 * M + 2], F32)  # [c_s(30) | c_y(30) | c_v | pad]
            alpha = small.tile([1, M], F32)
            u_row = small.tile([1, M], F32)
            nc.vector.memset(u_row, 0.0)
            tmp_r = small.tile([1, M], F32)
            dotn = small.tile([1, 1], F32)
            tsc = small.tile([1, 1], F32)

            def sy(j):  # s_j . y_k row
                return gf[:, j * W : j * W + M]

            def yy(j):
                return gf[:, (M + j) * W : (M + j) * W + M]

            sv = lambda j: gf[:, j * W + M : j * W + M + 1]
            yv = lambda j: gf[:, (M + j) * W + M : (M + j) * W + M + 1]

            # loop 1: alpha_j = (a*Sv_j - sum_{k>j} SY[j,k] alpha_k) / ys_j
            for j in range(M - 1, -1, -1):
                if j < M - 1:
                    nk = M - 1 - j
                    nc.vector.tensor_tensor(
                        out=tmp_r[:, :nk],
                        in0=gf[:, j * W + j + 1 : j * W + M],
                        in1=alpha[:, j + 1 : M],
                        op=mult,
                    )
                    nc.vector.tensor_reduce(
                        out=dotn, in_=tmp_r[:, :nk],
                        axis=mybir.AxisListType.X, op=add, negate=True,
                    )
                    nc.vector.scalar_tensor_tensor(
                        out=tsc, in0=sv(j), scalar=a_t, in1=dotn, op0=mult, op1=add
                    )
                else:
                    nc.vector.tensor_scalar(
                        out=tsc, in0=sv(j), scalar1=a_t, scalar2=None, op0=mult
                    )
                nc.vector.tensor_tensor(
                    out=alpha[:, j : j + 1], in0=tsc, in1=inv_ys[:, j : j + 1], op=mult
                )

            # loop 2: beta_j = (w_j/theta + u_j) / ys_j ;  d_j = alpha_j - beta_j
            # w_j = a*Yv_j - sum_k YY[j,k] alpha_k ;  u accumulates d_k * SY[k, :]
            for j in range(M):
                nc.vector.tensor_tensor(out=tmp_r, in0=yy(j), in1=alpha, op=mult)
                nc.vector.tensor_reduce(
                    out=dotn, in_=tmp_r, axis=mybir.AxisListType.X, op=add, negate=True
                )
                nc.vector.scalar_tensor_tensor(
                    out=tsc, in0=yv(j), scalar=a_t, in1=dotn, op0=mult, op1=add
                )
                nc.vector.scalar_tensor_tensor(
                    out=tsc, in0=tsc, scalar=inv_th, in1=u_row[:, j : j + 1],
                    op0=mult, op1=add,
                )
                nc.vector.tensor_tensor(
                    out=tsc, in0=tsc, in1=inv_ys[:, j : j + 1], op=mult
                )  # beta_j
                nc.vector.tensor_tensor(
                    out=coeff[:, j : j + 1], in0=alpha[:, j : j + 1], in1=tsc,
                    op=subtract,
                )  # d_j = c_s[j]
                if j < M - 1:
                    nc.vector.scalar_tensor_tensor(
                        out=u_row, in0=sy(j), scalar=coeff[:, j : j + 1], in1=u_row,
                        op0=mult, op1=add,
                    )

            # c_y = -alpha/theta ; c_v = a/theta
            nc.vector.tensor_scalar(
                out=coeff[:, M : 2 * M], in0=alpha, scalar1=inv_th, scalar2=-1.0,
                op0=mult, op1=mult,
            )
            nc.vector.tensor_scalar(
                out=coeff[:, 2 * M : 2 * M + 1], in0=a_t, scalar1=inv_th,
                scalar2=None, op0=mult,
            )

            # broadcast coeff row to all 128 partitions: ones[128]^T outer coeff
            ones_t = consts.tile([1, 128], F32)
            nc.vector.memset(ones_t, 1.0)
            cb_ps = psg_pool.tile([128, X], F32, tag="cbps")
            nc.tensor.matmul(cb_ps, ones_t, coeff[:, :X], start=True, stop=True)
            c_full = small.tile([128, X], F32)
            nc.any.tensor_copy(c_full, cb_ps)

            # ---------------- phase D: res = sum_j coeff_j * row_j ----------------
            for c in range(n_chunks_d):
                n0 = c * ch_d
                acc = dacc_pool.tile([128, f_d], F32, tag="dacc")
                vch = dv_pool.tile([128, f_d], F32, tag="dv")
                nc.sync.dma_start(
                    out=vch,
                    in_=v_d[n0 : n0 + ch_d].rearrange("(p f) -> p f", p=128),
                )
                nc.vector.tensor_scalar(
                    out=acc, in0=vch, scalar1=c_full[:, X - 1 : X], scalar2=None,
                    op0=mult,
                )
                for src, coff in ((s_d, 0), (y_d, M)):
                    for j in range(M):
                        row = drow_pool.tile([128, f_d], F32, tag="drow")
                        nc.sync.dma_start(
                            out=row,
                            in_=src[j, n0 : n0 + ch_d].rearrange(
                                "(p f) -> p f", p=128
                            ),
                        )
                        nc.vector.scalar_tensor_tensor(
                            out=acc, in0=row, scalar=c_full[:, coff + j : coff + j + 1],
                            in1=acc, op0=mult, op1=add,
                        )
                nc.sync.dma_start(
                    out=out_d[n0 : n0 + ch_d].rearrange("(p f) -> p f", p=128),
                    in_=acc,
                )

    nc.compile()
    return nc


_BUILD_CACHE = {}


def _get_nc(n_core: int, n_cores: int):
    key = (n_core, n_cores)
    if key not in _BUILD_CACHE:
        _BUILD_CACHE[key] = build_kernel(n_core, n_cores)
    return _BUILD_CACHE[key]


def run(v, s, y, ys, theta, a, trace=False):
    n = v.shape[0]
    n_core = n // NCORES
    nc = _get_nc(n_core, NCORES)
    in_maps = []
    for c in range(NCORES):
        sl = slice(c * n_core, (c + 1) * n_core)
        in_maps.append(
            {
                "s_s": np.ascontiguousarray(s[:, sl]),
                "y_s": np.ascontiguousarray(y[:, sl]),
                "v_s": np.ascontiguousarray(v[sl]),
                "ys": np.ascontiguousarray(ys),
                "theta": np.asarray(theta, dtype=np.float32).reshape(1),
                "a": np.asarray(a, dtype=np.float32).reshape(1),
            }
        )
    res = run_bass_kernel_spmd(nc, in_maps, list(range(NCORES)), trace=trace)
    out = np.concatenate([res.results[c]["out"] for c in range(NCORES)])
    return out, res


def kernel(v, s, y, ys, theta, a):
    out, _ = run(
        np.asarray(v, np.float32),
        np.asarray(s, np.float32),
        np.asarray(y, np.float32),
        np.asarray(ys, np.float32),
        theta,
        a,
    )
    return out



# revision 42
# speedup vs baseline: 1.1224x; 1.0270x over previous
"""L-BFGS two-loop recursion (apply_Hv) on 8 Trainium2 NeuronCores.

Vector-free reformulation: instead of 60 sequential dot-product/axpy steps
(each of which would need its own scalar AllReduce at a ~10us floor), the
two-loop recursion is algebraically equivalent to

  1. Gram pass   : G = [S; Y; v] @ [Y; v]^T            (one streaming pass)
  2. tiny scalar : alpha/beta recursions on the 61x31 Gram entries
  3. combine pass: res = (a/theta) v - sum_j (alpha_j/theta) y_j
                       + sum_j (alpha_j - beta_j) s_j  (one streaming pass)

The n dimension is sharded across the 8 cores; one 7.6 KB AllReduce merges
the per-core partial Gram matrices.  Both passes are HBM-bandwidth bound.

Per core, phase A streams natural-layout [122, F] tiles (rows = S(30), Y(30),
v packed twice along partitions for full DMA width), transposes 128-column
blocks on the TensorEngine (fp32 identity-matmul path), copies PSUM->SBUF in
bf16, and runs the Gram matmuls in bf16 (4x the fp32 PE rate) accumulating
in fp32 PSUM.  Phase D is a per-row fused scalar_tensor_tensor accumulation
on the VectorEngine.
"""

import numpy as np

import concourse.bass as bass
import concourse.mybir as mybir
from concourse import bacc
from concourse.bass_utils import run_bass_kernel_spmd
from concourse.masks import make_identity
from concourse.tile import TileContext

F32 = mybir.dt.float32
BF16 = mybir.dt.bfloat16
M = 30  # L-BFGS history length
X = 2 * M + 1  # rows of [S; Y; v]
NCORES = 8
N_FULL = 4_194_304
N_CORE = N_FULL // NCORES

# phase A: one chunk covers CH_A consecutive n per core, laid out as
# [122, F_A] (two n-halves stacked along partitions)
F_A = 4096
CH_A = 2 * F_A


def build_kernel(n_core: int = N_CORE, n_cores: int = NCORES):
    assert n_core % CH_A == 0
    n_chunks_a = n_core // CH_A
    # phase D: [128, f_d] tiles, n-chunk = 128 * f_d
    f_d = min(2048, n_core // 128)
    ch_d = 128 * f_d
    assert n_core % ch_d == 0
    n_chunks_d = n_core // ch_d

    nc = bacc.Bacc(None, target_bir_lowering=False, debug=False)

    s_d = nc.declare_dram_parameter("s_s", [M, n_core], F32, isOutput=False)
    y_d = nc.declare_dram_parameter("y_s", [M, n_core], F32, isOutput=False)
    v_d = nc.declare_dram_parameter("v_s", [n_core], F32, isOutput=False)
    ys_d = nc.declare_dram_parameter("ys", [M], F32, isOutput=False)
    th_d = nc.declare_dram_parameter("theta", [1], F32, isOutput=False)
    a_d = nc.declare_dram_parameter("a", [1], F32, isOutput=False)
    out_d = nc.declare_dram_parameter("out", [n_core], F32, isOutput=True)

    g_loc = nc.dram_tensor("g_loc", [X, M + 1], F32)
    g_red = nc.dram_tensor("g_red", [X, M + 1], F32, addr_space="Shared")

    add = mybir.AluOpType.add
    mult = mybir.AluOpType.mult
    subtract = mybir.AluOpType.subtract

    with TileContext(nc) as tc:
        with (
            tc.tile_pool(name="consts", bufs=1) as consts,
            tc.tile_pool(name="x2", bufs=3) as x2_pool,
            tc.tile_pool(name="xt", bufs=4) as xt_pool,
            tc.tile_pool(name="pstr", bufs=3, space="PSUM") as pstr_pool,
            tc.tile_pool(name="psg", bufs=2, space="PSUM") as psg_pool,
            tc.tile_pool(name="small", bufs=1) as small,
            tc.tile_pool(name="dacc", bufs=2) as dacc_pool,
            tc.tile_pool(name="drow", bufs=10) as drow_pool,
            tc.tile_pool(name="dv", bufs=2) as dv_pool,
        ):
            identity = consts.tile([122, 122], F32)
            make_identity(nc, identity)

            # ---------------- phase A: Gram matrix ----------------
            g_acc = small.tile([X, M + 1], F32)
            nc.vector.memset(g_acc, 0.0)

            for c in range(n_chunks_a):
                n0 = c * CH_A
                x2 = x2_pool.tile([2 * X, F_A], F32, tag="x2")
                # partition p = h*61 + j holds row j of [S;Y;v], n-half h.
                # plain partition-range slices only (nested partition dims in
                # one SBUF DMA dst are not supported).
                for h in range(2):
                    nh = n0 + h * F_A
                    nc.sync.dma_start(
                        out=x2[h * X : h * X + M, :],
                        in_=s_d[:, nh : nh + F_A],
                    )
                    nc.sync.dma_start(
                        out=x2[h * X + M : h * X + 2 * M, :],
                        in_=y_d[:, nh : nh + F_A],
                    )
                    nc.sync.dma_start(
                        out=x2[h * X + 2 * M : h * X + X, :],
                        in_=v_d[nh : nh + F_A].rearrange("(o f) -> o f", o=1),
                    )

                gps = psg_pool.tile([X, M + 1], F32, tag="gps")
                n_blk = F_A // 128  # 128-column transpose blocks
                mm = 0
                for q in range(n_blk // 4):
                    ps = pstr_pool.tile([128, 4, 2 * X], F32, tag="pstr")
                    for t in range(4):
                        b = q * 4 + t
                        nc.tensor.transpose(
                            ps[:, t, :], x2[:, b * 128 : (b + 1) * 128], identity
                        )
                    xt = xt_pool.tile([128, 4, 2 * X], BF16, tag="xt")
                    nc.any.tensor_copy(xt, ps)
                    for t in range(4):
                        for h in range(2):
                            nc.tensor.matmul(
                                gps,
                                xt[:, t, h * X : (h + 1) * X],
                                xt[:, t, h * X + M : (h + 1) * X],
                                start=(mm == 0),
                                stop=(mm == 8 * (n_blk // 4) - 1),
                            )
                            mm += 1
                nc.vector.tensor_tensor(out=g_acc, in0=g_acc, in1=gps, op=add)

            # ---------------- phase B: AllReduce ----------------
            nc.sync.dma_start(out=g_loc[:, :], in_=g_acc)
            nc.gpsimd.collective_compute(
                "AllReduce",
                add,
                ins=[g_loc[:, :]],
                outs=[g_red[:, :]],
                replica_groups=[list(range(n_cores))],
            )

            # ---------------- phase C: scalar recursions ----------------
            # everything on partition 0; G flattened to [1, X*(M+1)]
            W = M + 1
            gf = small.tile([1, X * W], F32)
            nc.sync.dma_start(
                out=gf, in_=g_red[:, :].rearrange("(o a) b -> o (a b)", o=1)
            )
            ys_t = small.tile([1, M], F32)
            nc.sync.dma_start(out=ys_t, in_=ys_d[:].rearrange("(o a) -> o a", o=1))
            a_t = small.tile([1, 1], F32)
            nc.sync.dma_start(out=a_t, in_=a_d[:].rearrange("(o a) -> o a", o=1))
            th_t = small.tile([1, 1], F32)
            nc.sync.dma_start(out=th_t, in_=th_d[:].rearrange("(o a) -> o a", o=1))

            inv_ys = small.tile([1, M], F32)
            nc.vector.reciprocal(inv_ys, ys_t)
            inv_th = small.tile([1, 1], F32)
            nc.vector.reciprocal(inv_th, th_t)

            coeff = small.tile([1, 2 * M + 2], F32)  # [c_s(30) | c_y(30) | c_v | pad]
            alpha = small.tile([1, M], F32)
            u_row = small.tile([1, M], F32)
            nc.vector.memset(u_row, 0.0)
            tmp_r = small.tile([1, M], F32)
            dotn = small.tile([1, 1], F32)
            tsc = small.tile([1, 1], F32)

            def sy(j):  # s_j . y_k row
                return gf[:, j * W : j * W + M]

            def yy(j):
                return gf[:, (M + j) * W : (M + j) * W + M]

            sv = lambda j: gf[:, j * W + M : j * W + M + 1]
            yv = lambda j: gf[:, (M + j) * W + M : (M + j) * W + M + 1]

            # loop 1: alpha_j = (a*Sv_j - sum_{k>j} SY[j,k] alpha_k) / ys_j
            for j in range(M - 1, -1, -1):
                if j < M - 1:
                    nk = M - 1 - j
                    nc.vector.tensor_tensor(
                        out=tmp_r[:, :nk],
                        in0=gf[:, j * W + j + 1 : j * W + M],
                        in1=alpha[:, j + 1 : M],
                        op=mult,
                    )
                    nc.vector.tensor_reduce(
                        out=dotn, in_=tmp_r[:, :nk],
                        axis=mybir.AxisListType.X, op=add, negate=True,
                    )
                    nc.vector.scalar_tensor_tensor(
                        out=tsc, in0=sv(j), scalar=a_t, in1=dotn, op0=mult, op1=add
                    )
                else:
                    nc.vector.tensor_scalar(
                        out=tsc, in0=sv(j), scalar1=a_t, scalar2=None, op0=mult
                    )
                nc.vector.tensor_tensor(
                    out=alpha[:, j : j + 1], in0=tsc, in1=inv_ys[:, j : j + 1], op=mult
                )

            # loop 2: beta_j = (w_j/theta + u_j) / ys_j ;  d_j = alpha_j - beta_j
            # w_j = a*Yv_j - sum_k YY[j,k] alpha_k ;  u accumulates d_k * SY[k, :]
            for j in range(M):
                nc.vector.tensor_tensor(out=tmp_r, in0=yy(j), in1=alpha, op=mult)
                nc.vector.tensor_reduce(
                    out=dotn, in_=tmp_r, axis=mybir.AxisListType.X, op=add, negate=True
                )
                nc.vector.scalar_tensor_tensor(
                    out=tsc, in0=yv(j), scalar=a_t, in1=dotn, op0=mult, op1=add
                )
                nc.vector.scalar_tensor_tensor(
                    out=tsc, in0=tsc, scalar=inv_th, in1=u_row[:, j : j + 1],
                    op0=mult, op1=add,
                )
                nc.vector.tensor_tensor(
                    out=tsc, in0=tsc, in1=inv_ys[:, j : j + 1], op=mult
                )  # beta_j
                nc.vector.tensor_tensor(
                    out=coeff[:, j : j + 1], in0=alpha[:, j : j + 1], in1=tsc,
                    op=subtract,
                )  # d_j = c_s[j]
                if j < M - 1:
                    nc.vector.scalar_tensor_tensor(
                        out=u_row, in0=sy(j), scalar=coeff[:, j : j + 1], in1=u_row,
                        op0=mult, op1=add,
                    )

            # c_y = -alpha/theta ; c_v = a/theta
            nc.vector.tensor_scalar(
                out=coeff[:, M : 2 * M], in0=alpha, scalar1=inv_th, scalar2=-1.0,
                op0=mult, op1=mult,
            )
            nc.vector.tensor_scalar(
                out=coeff[:, 2 * M : 2 * M + 1], in0=a_t, scalar1=inv_th,
                scalar2=None, op0=mult,
            )

            # broadcast coeff row to all 128 partitions: ones[128]^T outer coeff
            ones_t = consts.tile([1, 128], F32)
            nc.vector.memset(ones_t, 1.0)
            cb_ps = psg_pool.tile([128, X], F32, tag="cbps")
            nc.tensor.matmul(cb_ps, ones_t, coeff[:, :X], start=True, stop=True)
            c_full = small.tile([128, X], F32)
            nc.any.tensor_copy(c_full, cb_ps)

            # ---------------- phase D: res = sum_j coeff_j * row_j ----------------
            for c in range(n_chunks_d):
                n0 = c * ch_d
                acc = dacc_pool.tile([128, f_d], F32, tag="dacc")
                vch = dv_pool.tile([128, f_d], F32, tag="dv")
                nc.sync.dma_start(
                    out=vch,
                    in_=v_d[n0 : n0 + ch_d].rearrange("(p f) -> p f", p=128),
                )
                nc.vector.tensor_scalar(
                    out=acc, in0=vch, scalar1=c_full[:, X - 1 : X], scalar2=None,
                    op0=mult,
                )
                for src, coff in ((s_d, 0), (y_d, M)):
                    for j in range(M):
                        row = drow_pool.tile([128, f_d], F32, tag="drow")
                        nc.sync.dma_start(
                            out=row,
                            in_=src[j, n0 : n0 + ch_d].rearrange(
                                "(p f) -> p f", p=128
                            ),
                        )
                        nc.vector.scalar_tensor_tensor(
                            out=acc, in0=row, scalar=c_full[:, coff + j : coff + j + 1],
                            in1=acc, op0=mult, op1=add,
                        )
                nc.sync.dma_start(
                    out=out_d[n0 : n0 + ch_d].rearrange("(p f) -> p f", p=128),
                    in_=acc,
                )

    nc.compile()
    return nc


_BUILD_CACHE = {}


def _get_nc(n_core: int, n_cores: int):
    key = (n_core, n_cores)
    if key not in _BUILD_CACHE:
        _BUILD_CACHE[key] = build_kernel(n_core, n_cores)
    return _BUILD_CACHE[key]


def run(v, s, y, ys, theta, a, trace=False):
    n = v.shape[0]
    n_core = n // NCORES
    nc = _get_nc(n_core, NCORES)
    in_maps = []
    for c in range(NCORES):
        sl = slice(c * n_core, (c + 1) * n_core)
        in_maps.append(
            {
                "s_s": np.ascontiguousarray(s[:, sl]),
                "y_s": np.ascontiguousarray(y[:, sl]),
                "v_s": np.ascontiguousarray(v[sl]),
                "ys": np.ascontiguousarray(ys),
                "theta": np.asarray(theta, dtype=np.float32).reshape(1),
                "a": np.asarray(a, dtype=np.float32).reshape(1),
            }
        )
    res = run_bass_kernel_spmd(nc, in_maps, list(range(NCORES)), trace=trace)
    out = np.concatenate([res.results[c]["out"] for c in range(NCORES)])
    return out, res


def kernel(v, s, y, ys, theta, a):
    out, _ = run(
        np.asarray(v, np.float32),
        np.asarray(s, np.float32),
        np.asarray(y, np.float32),
        np.asarray(ys, np.float32),
        theta,
        a,
    )
    return out



# revision 43
# speedup vs baseline: 1.1358x; 1.0119x over previous
"""L-BFGS two-loop recursion (apply_Hv) on 8 Trainium2 NeuronCores.

Vector-free reformulation: instead of 60 sequential dot-product/axpy steps
(each of which would need its own scalar AllReduce at a ~10us floor), the
two-loop recursion is algebraically equivalent to

  1. Gram pass   : G = [S; Y; v] @ [Y; v]^T            (one streaming pass)
  2. tiny scalar : alpha/beta recursions on the 61x31 Gram entries
  3. combine pass: res = (a/theta) v - sum_j (alpha_j/theta) y_j
                       + sum_j (alpha_j - beta_j) s_j  (one streaming pass)

The n dimension is sharded across the 8 cores; one 7.6 KB AllReduce merges
the per-core partial Gram matrices.  Both passes are HBM-bandwidth bound.

Per core, phase A streams natural-layout [122, F] tiles (rows = S(30), Y(30),
v packed twice along partitions for full DMA width), transposes 128-column
blocks on the TensorEngine (fp32 needs the identity-matmul path), and
accumulates the Gram matrix in PSUM.  Phase D is a per-row fused
scalar_tensor_tensor accumulation on the VectorEngine.
"""

import numpy as np

import concourse.bass as bass
import concourse.mybir as mybir
from concourse import bacc
from concourse.bass_utils import run_bass_kernel_spmd
from concourse.masks import make_identity
from concourse.tile import TileContext

F32 = mybir.dt.float32
BF16 = mybir.dt.bfloat16
M = 30  # L-BFGS history length
X = 2 * M + 1  # rows of [S; Y; v]
NCORES = 8
N_FULL = 4_194_304
N_CORE = N_FULL // NCORES

# phase A: one chunk covers CH_A consecutive n per core, laid out as
# [122, F_A] (two n-halves stacked along partitions)
F_A = 4096
CH_A = 2 * F_A


def build_kernel(n_core: int = N_CORE, n_cores: int = NCORES):
    assert n_core % CH_A == 0
    n_chunks_a = n_core // CH_A
    # phase D: [128, f_d] tiles, n-chunk = 128 * f_d
    f_d = min(2048, n_core // 128)
    ch_d = 128 * f_d
    assert n_core % ch_d == 0
    n_chunks_d = n_core // ch_d

    nc = bacc.Bacc(None, target_bir_lowering=False, debug=False)

    s_d = nc.declare_dram_parameter("s_s", [M, n_core], F32, isOutput=False)
    y_d = nc.declare_dram_parameter("y_s", [M, n_core], F32, isOutput=False)
    v_d = nc.declare_dram_parameter("v_s", [n_core], F32, isOutput=False)
    ys_d = nc.declare_dram_parameter("ys", [M], F32, isOutput=False)
    th_d = nc.declare_dram_parameter("theta", [1], F32, isOutput=False)
    a_d = nc.declare_dram_parameter("a", [1], F32, isOutput=False)
    out_d = nc.declare_dram_parameter("out", [n_core], F32, isOutput=True)

    g_loc = nc.dram_tensor("g_loc", [X, M + 1], F32)
    g_red = nc.dram_tensor("g_red", [X, M + 1], F32, addr_space="Shared")

    add = mybir.AluOpType.add
    mult = mybir.AluOpType.mult
    subtract = mybir.AluOpType.subtract

    with TileContext(nc) as tc:
        with (
            tc.tile_pool(name="consts", bufs=1) as consts,
            tc.tile_pool(name="x2", bufs=3) as x2_pool,
            tc.tile_pool(name="xb", bufs=2) as xb_pool,
            tc.tile_pool(name="xt", bufs=4) as xt_pool,
            tc.tile_pool(name="pstr", bufs=3, space="PSUM") as pstr_pool,
            tc.tile_pool(name="psg", bufs=2, space="PSUM") as psg_pool,
            tc.tile_pool(name="small", bufs=1) as small,
            tc.tile_pool(name="dacc", bufs=2) as dacc_pool,
            tc.tile_pool(name="drow", bufs=8) as drow_pool,
            tc.tile_pool(name="dv", bufs=2) as dv_pool,
        ):
            id32 = consts.tile([122, 122], F32)
            make_identity(nc, id32)
            identity = consts.tile([122, 122], BF16)
            nc.vector.tensor_copy(identity, id32)

            # ---------------- phase A: Gram matrix ----------------
            g_acc = small.tile([X, M + 1], F32)
            nc.vector.memset(g_acc, 0.0)

            for c in range(n_chunks_a):
                n0 = c * CH_A
                x2 = x2_pool.tile([2 * X, F_A], F32, tag="x2")
                # partition p = h*61 + j holds row j of [S;Y;v], n-half h.
                # plain partition-range slices only (nested partition dims in
                # one SBUF DMA dst are not supported).
                for h in range(2):
                    nh = n0 + h * F_A
                    nc.sync.dma_start(
                        out=x2[h * X : h * X + M, :],
                        in_=s_d[:, nh : nh + F_A],
                    )
                    nc.sync.dma_start(
                        out=x2[h * X + M : h * X + 2 * M, :],
                        in_=y_d[:, nh : nh + F_A],
                    )
                    nc.sync.dma_start(
                        out=x2[h * X + 2 * M : h * X + X, :],
                        in_=v_d[nh : nh + F_A].rearrange("(o f) -> o f", o=1),
                    )

                # bf16 cast on the (otherwise idle) DVE: transposes and
                # their weight loads then run at 1 cyc/col instead of 2
                xb = xb_pool.tile([2 * X, F_A], BF16, tag="xb")
                nc.vector.tensor_copy(xb, x2)

                gps = psg_pool.tile([X, M + 1], F32, tag="gps")
                n_blk = F_A // 128  # 128-column transpose blocks
                mm = 0
                for q in range(n_blk // 4):
                    ps = pstr_pool.tile([128, 4, 2 * X], BF16, tag="pstr")
                    for t in range(4):
                        b = q * 4 + t
                        nc.tensor.transpose(
                            ps[:, t, :], xb[:, b * 128 : (b + 1) * 128], identity
                        )
                    xt = xt_pool.tile([128, 4, 2 * X], BF16, tag="xt")
                    nc.any.tensor_copy(xt, ps)
                    for t in range(4):
                        for h in range(2):
                            nc.tensor.matmul(
                                gps,
                                xt[:, t, h * X : (h + 1) * X],
                                xt[:, t, h * X + M : (h + 1) * X],
                                start=(mm == 0),
                                stop=(mm == 8 * (n_blk // 4) - 1),
                            )
                            mm += 1
                nc.vector.tensor_tensor(out=g_acc, in0=g_acc, in1=gps, op=add)

            # ---------------- phase B: AllReduce ----------------
            nc.sync.dma_start(out=g_loc[:, :], in_=g_acc)
            nc.gpsimd.collective_compute(
                "AllReduce",
                add,
                ins=[g_loc[:, :]],
                outs=[g_red[:, :]],
                replica_groups=[list(range(n_cores))],
            )

            # ---------------- phase C: scalar recursions ----------------
            # everything on partition 0; G flattened to [1, X*(M+1)]
            W = M + 1
            gf = small.tile([1, X * W], F32)
            nc.sync.dma_start(
                out=gf, in_=g_red[:, :].rearrange("(o a) b -> o (a b)", o=1)
            )
            ys_t = small.tile([1, M], F32)
            nc.sync.dma_start(out=ys_t, in_=ys_d[:].rearrange("(o a) -> o a", o=1))
            a_t = small.tile([1, 1], F32)
            nc.sync.dma_start(out=a_t, in_=a_d[:].rearrange("(o a) -> o a", o=1))
            th_t = small.tile([1, 1], F32)
            nc.sync.dma_start(out=th_t, in_=th_d[:].rearrange("(o a) -> o a", o=1))

            inv_ys = small.tile([1, M], F32)
            nc.vector.reciprocal(inv_ys, ys_t)
            inv_th = small.tile([1, 1], F32)
            nc.vector.reciprocal(inv_th, th_t)

            coeff = small.tile([1, 2 * M + 2], F32)  # [c_s(30) | c_y(30) | c_v | pad]
            alpha = small.tile([1, M], F32)
            u_row = small.tile([1, M], F32)
            nc.vector.memset(u_row, 0.0)
            tmp_r = small.tile([1, M], F32)
            dotn = small.tile([1, 1], F32)
            tsc = small.tile([1, 1], F32)

            def sy(j):  # s_j . y_k row
                return gf[:, j * W : j * W + M]

            def yy(j):
                return gf[:, (M + j) * W : (M + j) * W + M]

            sv = lambda j: gf[:, j * W + M : j * W + M + 1]
            yv = lambda j: gf[:, (M + j) * W + M : (M + j) * W + M + 1]

            # loop 1: alpha_j = (a*Sv_j - sum_{k>j} SY[j,k] alpha_k) / ys_j
            for j in range(M - 1, -1, -1):
                if j < M - 1:
                    nk = M - 1 - j
                    nc.vector.tensor_tensor(
                        out=tmp_r[:, :nk],
                        in0=gf[:, j * W + j + 1 : j * W + M],
                        in1=alpha[:, j + 1 : M],
                        op=mult,
                    )
                    nc.vector.tensor_reduce(
                        out=dotn, in_=tmp_r[:, :nk],
                        axis=mybir.AxisListType.X, op=add, negate=True,
                    )
                    nc.vector.scalar_tensor_tensor(
                        out=tsc, in0=sv(j), scalar=a_t, in1=dotn, op0=mult, op1=add
                    )
                else:
                    nc.vector.tensor_scalar(
                        out=tsc, in0=sv(j), scalar1=a_t, scalar2=None, op0=mult
                    )
                nc.vector.tensor_tensor(
                    out=alpha[:, j : j + 1], in0=tsc, in1=inv_ys[:, j : j + 1], op=mult
                )

            # loop 2: beta_j = (w_j/theta + u_j) / ys_j ;  d_j = alpha_j - beta_j
            # w_j = a*Yv_j - sum_k YY[j,k] alpha_k ;  u accumulates d_k * SY[k, :]
            for j in range(M):
                nc.vector.tensor_tensor(out=tmp_r, in0=yy(j), in1=alpha, op=mult)
                nc.vector.tensor_reduce(
                    out=dotn, in_=tmp_r, axis=mybir.AxisListType.X, op=add, negate=True
                )
                nc.vector.scalar_tensor_tensor(
                    out=tsc, in0=yv(j), scalar=a_t, in1=dotn, op0=mult, op1=add
                )
                nc.vector.scalar_tensor_tensor(
                    out=tsc, in0=tsc, scalar=inv_th, in1=u_row[:, j : j + 1],
                    op0=mult, op1=add,
                )
                nc.vector.tensor_tensor(
                    out=tsc, in0=tsc, in1=inv_ys[:, j : j + 1], op=mult
                )  # beta_j
                nc.vector.tensor_tensor(
                    out=coeff[:, j : j + 1], in0=alpha[:, j : j + 1], in1=tsc,
                    op=subtract,
                )  # d_j = c_s[j]
                if j < M - 1:
                    nc.vector.scalar_tensor_tensor(
                        out=u_row, in0=sy(j), scalar=coeff[:, j : j + 1], in1=u_row,
                        op0=mult, op1=add,
                    )

            # c_y = -alpha/theta ; c_v = a/theta
            nc.vector.tensor_scalar(
                out=coeff[:, M : 2 * M], in0=alpha, scalar1=inv_th, scalar2=-1.0,
                op0=mult, op1=mult,
            )
            nc.vector.tensor_scalar(
                out=coeff[:, 2 * M : 2 * M + 1], in0=a_t, scalar1=inv_th,
                scalar2=None, op0=mult,
            )

            # broadcast coeff row to all 128 partitions: ones[128]^T outer coeff
            ones_t = consts.tile([1, 128], F32)
            nc.vector.memset(ones_t, 1.0)
            cb_ps = psg_pool.tile([128, X], F32, tag="cbps")
            nc.tensor.matmul(cb_ps, ones_t, coeff[:, :X], start=True, stop=True)
            c_full = small.tile([128, X], F32)
            nc.any.tensor_copy(c_full, cb_ps)

            # ---------------- phase D: res = sum_j coeff_j * row_j ----------------
            for c in range(n_chunks_d):
                n0 = c * ch_d
                acc = dacc_pool.tile([128, f_d], F32, tag="dacc")
                vch = dv_pool.tile([128, f_d], F32, tag="dv")
                nc.sync.dma_start(
                    out=vch,
                    in_=v_d[n0 : n0 + ch_d].rearrange("(p f) -> p f", p=128),
                )
                nc.vector.tensor_scalar(
                    out=acc, in0=vch, scalar1=c_full[:, X - 1 : X], scalar2=None,
                    op0=mult,
                )
                for src, coff in ((s_d, 0), (y_d, M)):
                    for j in range(M):
                        row = drow_pool.tile([128, f_d], F32, tag="drow")
                        nc.sync.dma_start(
                            out=row,
                            in_=src[j, n0 : n0 + ch_d].rearrange(
                                "(p f) -> p f", p=128
                            ),
                        )
                        nc.vector.scalar_tensor_tensor(
                            out=acc, in0=row, scalar=c_full[:, coff + j : coff + j + 1],
                            in1=acc, op0=mult, op1=add,
                        )
                nc.sync.dma_start(
                    out=out_d[n0 : n0 + ch_d].rearrange("(p f) -> p f", p=128),
                    in_=acc,
                )

    nc.compile()
    return nc


_BUILD_CACHE = {}


def _get_nc(n_core: int, n_cores: int):
    key = (n_core, n_cores)
    if key not in _BUILD_CACHE:
        _BUILD_CACHE[key] = build_kernel(n_core, n_cores)
    return _BUILD_CACHE[key]


def run(v, s, y, ys, theta, a, trace=False):
    n = v.shape[0]
    n_core = n // NCORES
    nc = _get_nc(n_core, NCORES)
    in_maps = []
    for c in range(NCORES):
        sl = slice(c * n_core, (c + 1) * n_core)
        in_maps.append(
            {
                "s_s": np.ascontiguousarray(s[:, sl]),
                "y_s": np.ascontiguousarray(y[:, sl]),
                "v_s": np.ascontiguousarray(v[sl]),
                "ys": np.ascontiguousarray(ys),
                "theta": np.asarray(theta, dtype=np.float32).reshape(1),
                "a": np.asarray(a, dtype=np.float32).reshape(1),
            }
        )
    res = run_bass_kernel_spmd(nc, in_maps, list(range(NCORES)), trace=trace)
    out = np.concatenate([res.results[c]["out"] for c in range(NCORES)])
    return out, res


def kernel(v, s, y, ys, theta, a):
    out, _ = run(
        np.asarray(v, np.float32),
        np.asarray(s, np.float32),
        np.asarray(y, np.float32),
        np.asarray(ys, np.float32),
        theta,
        a,
    )
    return out

